# revision 5
# baseline (speedup 1.0000x reference)
"""Trainium2 Bass kernel for nn_MultiHeadGATEAULayer (multi-head GAT layer).

Strategy (edge-parallel, sort-based segment softmax):
  * Host: sort edges by (target-node window, src-half), pad each
    (window, src-half) group to 128-edge tiles, shard whole windows across
    the 8 cores (balanced by tile count). Each core owns a contiguous node
    range -> no collectives needed.
  * Device phase 0: compute projected node tables in HBM
      V-table [N,384] bf16 rows = [h_v | lv | h_h | pad]
      U-table [nodes_k,256] bf16 rows = [h_u | lu | pad]  (core-local nodes)
    plus SBUF-resident h0wT = (X @ W0 @ Wout^T)^T for the core's nodes.
  * Device phase 1 (per 128-edge tile):
      - matmul projects edge features: PSUM += XeT_tile^T @ [We|We_a|Wg]
      - dma_gather V rows (by src) and U rows (by tgt), identity-matmuls
        accumulate them into the same PSUM -> [s | logits | v]
      - ACT copy-casts [s|logits] to SBUF (s rows stored to HBM)
      - ACT Lrelu(+Exp) -> p = exp(leaky_relu(logits))  (no max subtraction:
        logits are O(10), exp is safe in fp32, and alpha is scale-invariant)
      - DVE: one-hot(tgt_rel) tile; weighted = v * p (per-head broadcast)
      - matmuls accumulate per-window transposed [sum_p | agg]:
          sum_p^T[h,w]  += p^T oh ; agg^T[hd,w] += weighted^T oh
      - per window: recip(sum_p+1e-10), replicate per-head via matmul,
        alpha-scale agg, Wout matmul, add h0wT + bias, store out^T.
  * Host: inverse-permute s rows -> new_edge_feature; transpose/concat
    out^T slices -> new_final.
"""

import hashlib
import os
import sys
import threading

sys.path.insert(0, "/opt/trn_rl_repo")
sys.path.insert(0, "/root/.axon_site/_ro/trn_rl_repo")

from contextlib import ExitStack

import ml_dtypes
import numpy as np

import concourse.bass as bass
import concourse.tile as tile
from concourse import bacc, mybir

BF16 = mybir.dt.bfloat16
F32 = mybir.dt.float32
I16 = mybir.dt.int16

BF = ml_dtypes.bfloat16

P = 128          # partitions / tile edge count / node window
GOP = 4          # tiles per gather op (<=512 idxs per dma_gather @768B rows)
XE_CHUNK = 32    # Xe tiles per DMA chunk (1 MiB bf16)
NCORES = 8

_NEFF_CACHE_DIR = os.path.join(os.path.dirname(os.path.abspath(__file__)), ".neff_cache")
_cache_installed = False


def _install_neff_cache():
    """Cache walrus compiles by BIR hash so unchanged cores skip recompile."""
    global _cache_installed
    if _cache_installed:
        return
    _cache_installed = True
    os.makedirs(_NEFF_CACHE_DIR, exist_ok=True)
    import concourse.bass2jax as b2j

    orig = b2j.compile_bir_kernel

    def cached(bir_json, tmpdir, neff_name="file.neff"):
        h = hashlib.sha256(
            bir_json if isinstance(bir_json, bytes) else bir_json.encode()
        ).hexdigest()
        cpath = os.path.join(_NEFF_CACHE_DIR, h + ".neff")
        if os.path.exists(cpath):
            sg = os.path.join(tmpdir, "sg00")
            os.makedirs(sg, exist_ok=True)
            out = os.path.join(sg, neff_name)
            with open(cpath, "rb") as f, open(out, "wb") as g:
                g.write(f.read())
            return out
        path = orig(bir_json, tmpdir, neff_name)
        try:
            with open(path, "rb") as f, open(cpath, "wb") as g:
                g.write(f.read())
        except OSError:
            pass
        return path

    b2j.compile_bir_kernel = cached


# ---------------------------------------------------------------------------
# host-side data preparation
# ---------------------------------------------------------------------------

def _wrap_idx16(idx_flat):
    """dma_gather idx layout: idx i at [i % 16, i // 16], replicated x8."""
    n = idx_flat.shape[0]
    w = idx_flat.reshape(n // 16, 16).T.astype(np.int16)  # [16, n/16]
    return np.ascontiguousarray(np.tile(w, (8, 1)))        # [128, n/16]


def _prep(inputs, ncores):
    X = np.asarray(inputs["node_feature_matrix"], np.float32)
    Xe = np.asarray(inputs["edge_feature_matrix"], np.float32)
    ei = np.asarray(inputs["edge_index"]).astype(np.int64)
    emap = np.asarray(inputs["edge_map"]).astype(np.int64)
    N, Dn = X.shape
    E, De = Xe.shape
    H = int(np.asarray(inputs["a_proj_b"]).shape[0])
    assert Dn == 128 and De == 128 and H == 8

    tgt, src = ei[0], ei[1]

    n_win = -(-N // P)
    node_split = ((N + 1) // 2 // P) * P  # 128-aligned src-half split
    assert node_split < 32768 and (N - node_split) < 32768

    # sort: window-major, then src-half, then src (gather locality)
    win = tgt // P
    shalf = (src >= node_split).astype(np.int64)
    order = np.lexsort((src, shalf, win))
    s_tgt, s_src, s_half, s_win = tgt[order], src[order], shalf[order], win[order]
    s_xerow = emap[order]

    # group boundaries: (window, half) runs; pad each to a tile multiple
    keys = s_win * 2 + s_half
    bnd = np.flatnonzero(np.diff(keys)) + 1
    starts = np.concatenate(([0], bnd))
    ends = np.concatenate((bnd, [E]))

    # slot arrays (padded); build per (window,half) group
    slot_src, slot_tgtrel, slot_xerow, slot_orig, slot_win, slot_half = \
        [], [], [], [], [], []
    for s0, s1 in zip(starts, ends):
        cnt = s1 - s0
        padded = -(-cnt // P) * P
        w = int(s_win[s0])
        h = int(s_half[s0])
        half_base = h * node_split
        srcs = np.full(padded, half_base, np.int64)
        srcs[:cnt] = s_src[s0:s1]
        trel = np.full(padded, -1, np.int64)
        trel[:cnt] = s_tgt[s0:s1] - w * P
        xr = np.full(padded, -1, np.int64)
        xr[:cnt] = s_xerow[s0:s1]
        og = np.full(padded, -1, np.int64)
        og[:cnt] = order[s0:s1]
        slot_src.append(srcs)
        slot_tgtrel.append(trel)
        slot_xerow.append(xr)
        slot_orig.append(og)
        slot_win.append(np.full(padded, w, np.int64))
        slot_half.append(np.full(padded, h, np.int64))
    slot_src = np.concatenate(slot_src)
    slot_tgtrel = np.concatenate(slot_tgtrel)
    slot_xerow = np.concatenate(slot_xerow)
    slot_orig = np.concatenate(slot_orig)
    slot_win = np.concatenate(slot_win)
    slot_half = np.concatenate(slot_half)
    n_slots = slot_src.shape[0]
    n_tiles = n_slots // P
    tile_win = slot_win[::P].copy()
    tile_half = slot_half[::P].copy()

    # shard whole windows across cores, balancing tile counts
    win_first_tile = {}
    for t in range(n_tiles):
        win_first_tile.setdefault(int(tile_win[t]), t)
    wins_sorted = sorted(win_first_tile)
    # tiles per window
    wtiles = {w: int(np.sum(tile_win == w)) for w in wins_sorted}
    total_tiles = n_tiles
    per_core = []
    target = total_tiles / ncores
    acc, cur = 0, []
    for w in wins_sorted:
        cur.append(w)
        acc += wtiles[w]
        if acc >= target * (len(per_core) + 1) and len(per_core) < ncores - 1:
            per_core.append(cur)
            cur = []
    per_core.append(cur)
    while len(per_core) < ncores:
        per_core.append([])

    # uniform per-core dims
    core_tiles = [sum(wtiles[w] for w in ws) for ws in per_core]
    t_max = -(-max(max(core_tiles), 1) // GOP) * GOP
    nodes_max_w = max((len(ws) for ws in per_core), default=1)
    nodes_max_w = max(nodes_max_w, 1)
    nwin_max = nodes_max_w  # windows per core (<= this)
    nodes_pad = nwin_max * P

    # weight packs (bf16)
    f32 = np.float32
    Wv = np.asarray(inputs["Wv"], f32)
    Wu = np.asarray(inputs["Wu"], f32)
    We = np.asarray(inputs["We"], f32)
    Wh = np.asarray(inputs["Wh"], f32)
    Wg = np.asarray(inputs["Wg"], f32)
    W0 = np.asarray(inputs["W0"], f32)
    aW = np.asarray(inputs["a_proj_w"], f32)     # [H, De]
    ab = np.asarray(inputs["a_proj_b"], f32)     # [H]
    WoW = np.asarray(inputs["Wout_w"], f32)      # [Do, Do]
    Wob = np.asarray(inputs["Wout_b"], f32)      # [Do]

    WVP = np.concatenate([Wv, Wv @ aW.T, Wh], axis=1).astype(BF)      # [128,264]
    WUP = np.concatenate([Wu, Wu @ aW.T], axis=1).astype(BF)          # [128,136]
    WEP = np.concatenate([We, We @ aW.T, Wg], axis=1).astype(BF)      # [128,264]
    Wagg = np.ascontiguousarray(WoW.T).astype(BF)                     # [hd, j]
    W0W = (W0 @ WoW.T).astype(BF)                                     # [128,128]

    n_pad_full = n_win * P
    XT_full = np.zeros((P, n_pad_full), BF)
    XT_full[:, :N] = X.T.astype(BF)

    iota = np.tile(np.arange(P, dtype=np.float32), (P, 1))
    ident = np.eye(P, dtype=np.float32).astype(BF)
    repmat = np.zeros((P, P), np.float32)
    for h in range(H):
        repmat[h, h * (P // H):(h + 1) * (P // H)] = 1.0
    repmat = repmat.astype(BF)
    bias_col = Wob.reshape(P, 1).astype(np.float32)
    btile = np.tile(ab.reshape(1, H), (P, 1)).astype(np.float32)

    Xe_bf = Xe.astype(BF)

    cores = []
    t_cursor = 0
    for k in range(ncores):
        ws = per_core[k]
        ntk = core_tiles[k]
        t0, t1 = t_cursor, t_cursor + ntk
        t_cursor = t1
        sl = slice(t0 * P, t1 * P)

        # per-core slot arrays padded to t_max tiles
        srcs = np.zeros(t_max * P, np.int64)
        trel = np.full(t_max * P, -1.0, np.float32)
        xer = np.full(t_max * P, -1, np.int64)
        og = np.full(t_max * P, -1, np.int64)
        twin = np.zeros(t_max, np.int64)
        thalf = np.zeros(t_max, np.int64)
        srcs[: ntk * P] = slot_src[sl]
        trel[: ntk * P] = slot_tgtrel[sl].astype(np.float32)
        xer[: ntk * P] = slot_xerow[sl]
        og[: ntk * P] = slot_orig[sl]
        twin[:t_max] = ws[-1] if ws else 0
        twin[:ntk] = tile_win[t0:t1]
        thalf[:ntk] = tile_half[t0:t1]
        if ntk < t_max:  # dummy tiles extend the last real window, half 0
            srcs[ntk * P:] = 0
            thalf[ntk:] = 0

        node_base = (ws[0] * P) if ws else 0
        nodes_k = min(N, (ws[-1] + 1) * P) - node_base if ws else 0

        vidx = srcs - thalf.repeat(P) * node_split
        uidx_val = np.zeros(t_max * P, np.int64)
        real = trel >= 0
        # tgt for real slots; window base for pads (valid local row)
        uidx_val = twin.repeat(P) * P - node_base
        uidx_val = uidx_val + np.where(real, trel.astype(np.int64), 0)
        assert vidx.min() >= 0 and vidx.max() < 32768
        assert uidx_val.min() >= 0 and uidx_val.max() < nodes_pad

        # Xe transposed, permuted; zeros at pads
        xe_rows = np.zeros((t_max * P, P), BF)
        rmask = xer >= 0
        xe_rows[rmask] = Xe_bf[xer[rmask]]
        xe_pt = np.ascontiguousarray(xe_rows.T)  # [128, t_max*128]

        XT_local = np.zeros((P, nodes_pad), BF)
        end = min(node_base + nodes_pad, N)
        if ws:
            XT_local[:, : end - node_base] = X.T[:, node_base:end].astype(BF)

        # tgtrel in [128, t_max] layout (column t = tile t)
        trel_cols = np.ascontiguousarray(trel.reshape(t_max, P).T)

        # metadata
        tiles_meta = []
        for t in range(t_max):
            w = int(twin[t])
            wl = (w - (ws[0] if ws else 0))
            first = t == 0 or twin[t - 1] != w
            last = t == t_max - 1 or twin[t + 1] != w
            # dummy tiles attached to last window: they are never 'first'
            # unless the core is empty
            tiles_meta.append(dict(win_local=wl, first=first, last=last,
                                   vhalf=int(thalf[t])))
        # gather ops: runs of tiles with same vhalf, up to GOP
        gops = []
        t = 0
        while t < t_max:
            nt = 1
            while (nt < GOP and t + nt < t_max
                   and tiles_meta[t + nt]["vhalf"] == tiles_meta[t]["vhalf"]):
                nt += 1
            gops.append((t, nt, tiles_meta[t]["vhalf"]))
            t += nt

        n_win_k = len(ws) if ws else 1

        cores.append(dict(
            node_base=node_base, nodes_k=nodes_k, n_win_k=n_win_k,
            tiles_meta=tiles_meta, gops=gops, ntk=ntk,
            inmap=dict(
                xt_full=XT_full, xt_local=XT_local, xe_pt=xe_pt,
                vidx=_wrap_idx16(vidx), uidx=_wrap_idx16(uidx_val),
                tgtrel=trel_cols,
                wvp=WVP, wup=WUP, wep=WEP, wagg=Wagg, w0w=W0W,
                ident=ident, iota=iota, repmat=repmat, bias_col=bias_col,
                btile=btile,
            ),
            orig_ids=og,
        ))

    meta = dict(N=N, E=E, H=H, node_split=node_split, n_win=n_win,
                t_max=t_max, nodes_pad=nodes_pad, n_pad_full=n_pad_full,
                has_abias=bool(np.any(ab != 0.0)))
    return cores, meta


# ---------------------------------------------------------------------------
# device module builder (one core)
# ---------------------------------------------------------------------------

def build_module(core, meta):
    t_max = meta["t_max"]
    nodes_pad = meta["nodes_pad"]
    n_pad_full = meta["n_pad_full"]
    node_split = meta["node_split"]
    has_abias = meta["has_abias"]
    H = meta["H"]
    HD = P // H

    nc = bacc.Bacc("TRN2", target_bir_lowering=False, debug=False,
                   num_devices=1, num_swdge_queues=2)

    din = lambda n, s, d: nc.dram_tensor(n, s, d, kind="ExternalInput").ap()
    xt_full = din("xt_full", [P, n_pad_full], BF16)
    xt_local = din("xt_local", [P, nodes_pad], BF16)
    xe_pt = din("xe_pt", [P, t_max * P], BF16)
    vidx = din("vidx", [P, t_max * 8], I16)
    uidx = din("uidx", [P, t_max * 8], I16)
    tgtrel = din("tgtrel", [P, t_max], F32)
    wvp = din("wvp", [P, 264], BF16)
    wup = din("wup", [P, 136], BF16)
    wep = din("wep", [P, 264], BF16)
    wagg = din("wagg", [P, P], BF16)
    w0w = din("w0w", [P, P], BF16)
    ident = din("ident", [P, P], BF16)
    iota = din("iota", [P, P], F32)
    repmat = din("repmat", [P, P], BF16)
    bias_col = din("bias_col", [P, 1], F32)
    btile = din("btile", [P, H], F32)

    v_tab = nc.dram_tensor("v_tab", [n_pad_full, 384], BF16).ap()
    u_tab = nc.dram_tensor("u_tab", [nodes_pad, 256], BF16).ap()
    s_out = nc.dram_tensor("s_out", [t_max * P, P], BF16,
                           kind="ExternalOutput").ap()
    o_out = nc.dram_tensor("o_out", [P, nodes_pad], F32,
                           kind="ExternalOutput").ap()

    with tile.TileContext(nc) as tc, ExitStack() as ctx:
        cpool = ctx.enter_context(tc.tile_pool(name="consts", bufs=1))

        def cload(ap_in, shape, dt, tag):
            t = cpool.tile(shape, dt, tag=tag)
            nc.sync.dma_start(t[:], ap_in[:])
            return t

        wvp_s = cload(wvp, [P, 264], BF16, "wvp")
        wup_s = cload(wup, [P, 136], BF16, "wup")
        wep_s = cload(wep, [P, 264], BF16, "wep")
        wagg_s = cload(wagg, [P, P], BF16, "wagg")
        w0w_s = cload(w0w, [P, P], BF16, "w0w")
        ident_s = cload(ident, [P, P], BF16, "ident")
        iota_s = cload(iota, [P, P], F32, "iota")
        repmat_s = cload(repmat, [P, P], BF16, "repmat")
        bias_s = cload(bias_col, [P, 1], F32, "bias")
        btile_s = cload(btile, [P, H], F32, "btile")
        tgtrel_s = cload(tgtrel, [P, t_max], F32, "tgtrel")
        vidx_s = cload(vidx, [P, t_max * 8], I16, "vidx")
        uidx_s = cload(uidx, [P, t_max * 8], I16, "uidx")
        xtl_s = cload(xt_local, [P, nodes_pad], BF16, "xtl")

        h0w_s = cpool.tile([P, nodes_pad], F32, tag="h0w")

        # ---------------- phase 0: tables ----------------
        with ExitStack() as p0:
            xchunk_p = p0.enter_context(tc.tile_pool(name="p0xc", bufs=2))
            stg_p = p0.enter_context(tc.tile_pool(name="p0stg", bufs=3))
            ps_p = p0.enter_context(tc.tile_pool(name="p0ps", bufs=3,
                                                 space="PSUM"))

            # V-table over all nodes
            n_nt = n_pad_full // P
            CH = 4096
            for c0 in range(0, n_pad_full, CH):
                cw = min(CH, n_pad_full - c0)
                xc = xchunk_p.tile([P, CH], BF16, tag="xc")
                nc.sync.dma_start(xc[:, :cw], xt_full[:, c0:c0 + cw])
                for j in range(cw // P):
                    ps = ps_p.tile([P, 264], F32, tag="vps")
                    nc.tensor.matmul(out=ps[:], lhsT=xc[:, j * P:(j + 1) * P],
                                     rhs=wvp_s[:], start=True, stop=True)
                    stg = stg_p.tile([P, 384], BF16, tag="vstg")
                    nc.vector.memset(stg[:, 264:384], 0.0)
                    nc.scalar.copy(stg[:, 0:264], ps[:])
                    r0 = c0 + j * P
                    nc.sync.dma_start(v_tab[r0:r0 + P, :], stg[:])

            # U-table over local nodes
            for j in range(nodes_pad // P):
                ps = ps_p.tile([P, 264], F32, tag="vps")
                nc.tensor.matmul(out=ps[:, 0:136],
                                 lhsT=xtl_s[:, j * P:(j + 1) * P],
                                 rhs=wup_s[:], start=True, stop=True)
                stg = stg_p.tile([P, 384], BF16, tag="vstg")
                nc.vector.memset(stg[:, 136:256], 0.0)
                nc.scalar.copy(stg[:, 0:136], ps[:, 0:136])
                nc.sync.dma_start(u_tab[j * P:(j + 1) * P, :], stg[:, 0:256])

            # h0wT = W0W^T @ XT_local
            for c0 in range(0, nodes_pad, 512):
                cw = min(512, nodes_pad - c0)
                ps = ps_p.tile([P, 512], F32, tag="h0ps")
                nc.tensor.matmul(out=ps[:, :cw], lhsT=w0w_s[:],
                                 rhs=xtl_s[:, c0:c0 + cw],
                                 start=True, stop=True)
                nc.vector.tensor_copy(h0w_s[:, c0:c0 + cw], ps[:, :cw])

        # ---------------- phase 1: edges ----------------
        with ExitStack() as p1:
            xe_p = p1.enter_context(tc.tile_pool(name="xe", bufs=2))
            vg_p = p1.enter_context(tc.tile_pool(name="vg", bufs=3))
            ug_p = p1.enter_context(tc.tile_pool(name="ug", bufs=3))
            fin_p = p1.enter_context(tc.tile_pool(name="fin", bufs=3))
            ptmp_p = p1.enter_context(tc.tile_pool(name="ptmp", bufs=3))
            aggw_p = p1.enter_context(tc.tile_pool(name="aggw", bufs=3))
            aggp_p = p1.enter_context(tc.tile_pool(name="aggp", bufs=3))
            oh_p = p1.enter_context(tc.tile_pool(name="oh", bufs=4))
            eps_p = p1.enter_context(tc.tile_pool(name="eps", bufs=4,
                                                  space="PSUM"))
            aswin_p = p1.enter_context(tc.tile_pool(name="aswin", bufs=2,
                                                    space="PSUM"))
            fz_ps_p = p1.enter_context(tc.tile_pool(name="fzps", bufs=2,
                                                    space="PSUM"))
            fz_sb_p = p1.enter_context(tc.tile_pool(name="fzsb", bufs=2))

            tiles_meta = core["tiles_meta"]
            gops = core["gops"]

            cur_xe = None
            cur_xe_c = -1
            agg_ps = None
            sump_ps = None

            def finalize(win_local):
                nonlocal agg_ps, sump_ps
                t1 = fz_sb_p.tile([8, P], F32, tag="t1")
                nc.vector.tensor_scalar_add(t1[:], sump_ps[0:8, :], 1e-10)
                rec = fz_sb_p.tile([8, P], F32, tag="rec")
                nc.vector.reciprocal(rec[:], t1[:])
                rec_b = fz_sb_p.tile([8, P], BF16, tag="recb")
                nc.vector.tensor_copy(rec_b[:], rec[:])
                rep_ps = fz_ps_p.tile([P, P], F32, tag="fz")
                nc.tensor.matmul(out=rep_ps[:], lhsT=repmat_s[0:8, :],
                                 rhs=rec_b[:], start=True, stop=True)
                rep_sb = fz_sb_p.tile([P, P], F32, tag="repsb")
                nc.vector.tensor_copy(rep_sb[:], rep_ps[:])
                alpha_sb = fz_sb_p.tile([P, P], BF16, tag="alpha")
                nc.vector.tensor_tensor(out=alpha_sb[:], in0=agg_ps,
                                        in1=rep_sb[:],
                                        op=mybir.AluOpType.mult)
                out_ps = fz_ps_p.tile([P, P], F32, tag="fz")
                nc.tensor.matmul(out=out_ps[:], lhsT=wagg_s[:],
                                 rhs=alpha_sb[:], start=True, stop=True)
                out_sb = fz_sb_p.tile([P, P], F32, tag="osb")
                nc.vector.scalar_tensor_tensor(
                    out=out_sb[:], in0=out_ps[:], scalar=bias_s[:],
                    in1=h0w_s[:, win_local * P:(win_local + 1) * P],
                    op0=mybir.AluOpType.add, op1=mybir.AluOpType.add)
                nc.sync.dma_start(
                    o_out[:, win_local * P:(win_local + 1) * P], out_sb[:])
                agg_ps = None
                sump_ps = None

            for (t0, nt, vhalf) in gops:
                vg = vg_p.tile([P, GOP, 384], BF16, tag="vg")
                vbase = vhalf * node_split
                nc.gpsimd.dma_gather(
                    out_ap=vg[:, :nt, :],
                    in_ap=v_tab[vbase:vbase + (node_split if vhalf == 0
                                               else n_pad_full - vbase), :],
                    idxs_ap=vidx_s[:, t0 * 8:(t0 + nt) * 8],
                    num_idxs=nt * P, num_idxs_reg=nt * P,
                    elem_size=384, queue_num=0)
                ug = ug_p.tile([P, GOP, 256], BF16, tag="ug")
                nc.gpsimd.dma_gather(
                    out_ap=ug[:, :nt, :], in_ap=u_tab[:],
                    idxs_ap=uidx_s[:, t0 * 8:(t0 + nt) * 8],
                    num_idxs=nt * P, num_idxs_reg=nt * P,
                    elem_size=256, queue_num=1)

                fin = fin_p.tile([P, GOP, 136], BF16, tag="fin")
                aggw = aggw_p.tile([P, GOP, P], BF16, tag="aggw")
                aggp = aggp_p.tile([P, GOP, H], BF16, tag="aggp")
                pss = []
                for j in range(nt):
                    t = t0 + j
                    c = t // XE_CHUNK
                    if c != cur_xe_c:
                        cur_xe = xe_p.tile([P, XE_CHUNK * P], BF16, tag="xec")
                        c0 = c * XE_CHUNK * P
                        cw = min(XE_CHUNK * P, t_max * P - c0)
                        nc.sync.dma_start(cur_xe[:, :cw],
                                          xe_pt[:, c0:c0 + cw])
                        cur_xe_c = c
                    jj = t % XE_CHUNK
                    ps = eps_p.tile([P, 264], F32, tag="eps")
                    pss.append(ps)
                    nc.tensor.matmul(out=ps[:],
                                     lhsT=cur_xe[:, jj * P:(jj + 1) * P],
                                     rhs=wep_s[:], start=True, stop=False,
                                     skip_group_check=True)
                    nc.tensor.matmul(out=ps[:, 0:136], lhsT=ident_s[:],
                                     rhs=vg[:, j, 0:136], start=False,
                                     stop=False, skip_group_check=True)
                    nc.tensor.matmul(out=ps[:, 136:264], lhsT=ident_s[:],
                                     rhs=vg[:, j, 136:264], start=False,
                                     stop=True, skip_group_check=True)
                    nc.tensor.matmul(out=ps[:, 0:136], lhsT=ident_s[:],
                                     rhs=ug[:, j, 0:136], start=False,
                                     stop=True, skip_group_check=True)
                    nc.scalar.copy(fin[:, j, :], ps[:, 0:136])

                # p = exp(lrelu(logits + b))
                ptmp = ptmp_p.tile([P, GOP, H], F32, tag="pt")
                lg_in = fin[:, :nt, 128:136]
                if has_abias:
                    nc.vector.tensor_tensor(
                        out=ptmp[:, :nt, :], in0=lg_in,
                        in1=btile_s[:, 0:1, :]
                        if False else btile_s[:].broadcast_to([P, nt, H]),
                        op=mybir.AluOpType.add)
                    lg_in = ptmp[:, :nt, :]
                ptmp2 = ptmp_p.tile([P, GOP, H], F32, tag="pt2")
                nc.vector.scalar_tensor_tensor(
                    out=ptmp2[:, :nt, :], in0=lg_in, scalar=0.2,
                    in1=lg_in, op0=mybir.AluOpType.mult,
                    op1=mybir.AluOpType.max)
                nc.scalar.activation(aggp[:, :nt, :], ptmp2[:, :nt, :],
                                     mybir.ActivationFunctionType.Exp)

                for j in range(nt):
                    t = t0 + j
                    m = tiles_meta[t]
                    ps = pss[j]
                    oh = oh_p.tile([P, P], BF16, tag="oh")
                    nc.vector.tensor_scalar(
                        out=oh[:], in0=iota_s[:],
                        scalar1=tgtrel_s[:, t:t + 1], scalar2=None,
                        op0=mybir.AluOpType.is_equal)
                    nc.vector.tensor_tensor(
                        out=aggw[:, j, :], in0=ps[:, 136:264],
                        in1=aggp[:, j, :].broadcast_to([P, H, HD]),
                        op=mybir.AluOpType.mult)
                    if m["first"]:
                        asw = aswin_p.tile([P, 2 * P], F32, tag="asw")
                        agg_ps, sump_ps = asw[:, 0:P], asw[:, P:2 * P]
                    nc.tensor.matmul(out=agg_ps, lhsT=aggw[:, j, :],
                                     rhs=oh[:], start=m["first"],
                                     stop=m["last"], skip_group_check=True)
                    nc.tensor.matmul(out=sump_ps[0:H, :],
                                     lhsT=aggp[:, j, :], rhs=oh[:],
                                     start=False, stop=m["last"],
                                     skip_group_check=True)
                    if m["last"]:
                        finalize(m["win_local"])

                nc.sync.dma_start(
                    s_out[t0 * P:(t0 + nt) * P, :]
                    .rearrange("(t p) c -> p t c", p=P),
                    fin[:, :nt, 0:128])

    nc.compile()
    return nc


# ---------------------------------------------------------------------------
# runner
# ---------------------------------------------------------------------------

def _run_all(modules, inmaps, sequential=False):
    _install_neff_cache()
    import jax

    import concourse.bass2jax as b2j

    devices = jax.devices()
    results = [None] * len(modules)

    def go(k):
        with jax.default_device(devices[k % len(devices)]):
            results[k] = b2j.run_bass_via_pjrt(modules[k], [inmaps[k]],
                                               n_cores=1)[0]

    if sequential:
        for k in range(len(modules)):
            go(k)
    else:
        # compile serially (jax tracing is not re-entrant-safe here), then
        # dispatch is inside each call; threads give concurrent execution
        threads = [threading.Thread(target=go, args=(k,))
                   for k in range(len(modules))]
        for t in threads:
            t.start()
        for t in threads:
            t.join()
    return results


_BUILD_CACHE = {}


def build_all(inputs, ncores=NCORES):
    cores, meta = _prep(inputs, ncores)
    modules = []
    for k in range(ncores):
        modules.append(build_module(cores[k], meta))
    return cores, meta, modules


def kernel(**inputs):
    ncores = int(os.environ.get("GAT_CORES", NCORES))
    cores, meta, modules = build_all(inputs, ncores)
    inmaps = [c["inmap"] for c in cores]
    res = _run_all(modules, inmaps,
                   sequential=bool(int(os.environ.get("GAT_SEQ", "0"))))

    N, E = meta["N"], meta["E"]
    new_final = np.zeros((N, 128), np.float32)
    new_edge = np.zeros((E, 128), np.float32)
    for k in range(ncores):
        c = cores[k]
        if c["nodes_k"] > 0:
            nb = c["node_base"]
            new_final[nb:nb + c["nodes_k"], :] = \
                res[k]["o_out"][:, :c["nodes_k"]].T
        og = c["orig_ids"]
        rm = og >= 0
        new_edge[og[rm]] = res[k]["s_out"][rm].astype(np.float32)
    return new_final, new_edge


if __name__ == "__main__":
    pass
